# revision 2
# baseline (speedup 1.0000x reference)
"""Trainium2 Bass kernel for nn_Bert_AvgPooling (segment_reduce + mean + FC).

reference semantics:
    tokens = sequence_output.reshape(B*S, H)              # [32768, 768]
    sums   = segment_sum(tokens, seg_ids, 1537)           # sentinel id 1536
    mean   = sums[:1536] / clause_counts[:, None]
    logits = mean @ fc_w.T + fc_b                         # [1536, 16]

Strategy (8 cores, data parallel over tokens, NO collective):
  - Each core streams its 4096 tokens (12 MB f32) from HBM; the f32->bf16
    cast happens inside the SWDGE DMA engine.
  - Per 128-token tile a one-hot matrix (token x clause-window) is built on
    DVE with is_equal against an iota row, then PE matmul accumulates
    window sums in PSUM:  psum[c, h] += onehot.T @ tokens.
  - Clause ids are monotone over the token stream, so each group of R=16
    tiles shares one 128-wide clause window (host-verified; static 12-window
    full-sweep fallback for unsorted ids).
  - Per group: PSUM sums -> bf16 -> PE transpose -> FC matmul (768->16)
    -> scale by 1/count -> DMA the [128,16] window block to DRAM.
  - The host overlap-adds the 16 window blocks (windows of adjacent
    cores/groups share at most one boundary clause) and adds the bias once.
    This removes the ReduceScatter + arrival barrier (~65us) entirely.
"""

import sys

for _p in ("/opt/trn_rl_repo", "/opt/trn_rl_repo/concourse"):
    if _p not in sys.path:
        sys.path.insert(0, _p)

import numpy as np

import concourse.bacc as bacc
import concourse.bass as bass
import concourse.mybir as mybir
import concourse.tile as tile
from concourse import bass_utils
from concourse.masks import make_identity

F32 = mybir.dt.float32
BF16 = mybir.dt.bfloat16
I32 = mybir.dt.int32

B, S, H, NC = 64, 512, 768, 1536
CORES = 8
TPC = B * S // CORES  # tokens per core = 4096
NT = TPC // 128  # token tiles per core = 32
NCP = 1664  # padded clause rows (13 * 128)

# module-level stash for benchmarking (test.py reads these)
LAST_EXEC_INFO = {}


def _choose_groups(seg_flat):
    """Pick largest R in {32,16,8,4,2,1} s.t. every (core, group-of-R-tiles)
    clause-id span fits in a 128-wide window. Returns (R, bases[CORES][G])
    or (None, None) if even R=1 fails (=> general fallback path)."""
    ids = seg_flat.reshape(CORES, NT, 128)
    for R in (32, 16, 8, 4, 2, 1):
        G = NT // R
        bases = np.zeros((CORES, G), dtype=np.int64)
        ok = True
        for c in range(CORES):
            for g in range(G):
                grp = ids[c, g * R : (g + 1) * R].reshape(-1)
                real = grp[grp < NC]
                if real.size == 0:
                    bases[c, g] = 0
                    continue
                lo, hi = int(real.min()), int(real.max())
                if hi - lo > 127:
                    ok = False
                    break
                # clamp so the 128-row window stays inside the padded range
                bases[c, g] = min(lo, NCP - 128)
            if not ok:
                break
        if ok:
            return R, bases
    return None, None


def _build_fast(R, CH=4):
    """Collective-free grouped-window path. Same program for all cores;
    per-core data differences come in through input tensors."""
    G = NT // R
    nc = bacc.Bacc(
        "TRN2",
        target_bir_lowering=False,
        debug=False,
        enable_asserts=False,
        num_devices=CORES,
    )
    tok_d = nc.dram_tensor("tok", [TPC, H], F32, kind="ExternalInput")
    ids_d = nc.dram_tensor("ids", [128, NT], I32, kind="ExternalInput")
    base_d = nc.dram_tensor("base", [128, G], F32, kind="ExternalInput")
    cw_d = nc.dram_tensor("cw", [128, G], F32, kind="ExternalInput")
    iota_d = nc.dram_tensor("iota", [128, 128], F32, kind="ExternalInput")
    fcw_d = nc.dram_tensor("fcw", [128, 6, 16], F32, kind="ExternalInput")
    out_d = nc.dram_tensor("out", [G, 128, 16], F32, kind="ExternalOutput")

    from contextlib import ExitStack

    with tile.TileContext(nc) as tc, ExitStack() as ctx:
        cpool = ctx.enter_context(tc.tile_pool(name="const", bufs=1))
        iota_s = cpool.tile([128, 128], F32)
        nc.sync.dma_start(out=iota_s[:], in_=iota_d[:])
        ident = cpool.tile([128, 128], BF16)
        make_identity(nc, ident[:])
        fcw_f = cpool.tile([128, 6, 16], F32)
        nc.sync.dma_start(out=fcw_f[:], in_=fcw_d[:])
        fcw_s = cpool.tile([128, 6, 16], BF16)
        nc.vector.tensor_copy(fcw_s[:], fcw_f[:])
        ids_i = cpool.tile([128, NT], I32)
        nc.sync.dma_start(out=ids_i[:], in_=ids_d[:])
        ids_f = cpool.tile([128, NT], F32)
        nc.vector.tensor_copy(ids_f[:], ids_i[:])
        base_s = cpool.tile([128, G], F32)
        nc.sync.dma_start(out=base_s[:], in_=base_d[:])
        cw_s = cpool.tile([128, G], F32)
        nc.sync.dma_start(out=cw_s[:], in_=cw_d[:])
        invc = cpool.tile([128, G], F32)
        nc.vector.reciprocal(invc[:], cw_s[:])

        bfp = ctx.enter_context(tc.tile_pool(name="tokb", bufs=max(4, 48 // CH)))
        ohp = ctx.enter_context(tc.tile_pool(name="oh", bufs=8))
        smallp = ctx.enter_context(tc.tile_pool(name="small", bufs=8))
        evacp = ctx.enter_context(tc.tile_pool(name="evac", bufs=2))
        psA = ctx.enter_context(tc.tile_pool(name="psA", bufs=2, space="PSUM"))
        psT = ctx.enter_context(tc.tile_pool(name="psT", bufs=2, space="PSUM"))
        psF = ctx.enter_context(tc.tile_pool(name="psF", bufs=2, space="PSUM"))

        for g in range(G):
            ps = psA.tile([128, H], F32, tag="psA", space="PSUM")
            chunks = [(g * R + c0, min(CH, R - c0)) for c0 in range(0, R, CH)]
            if g == G - 1 and chunks[-1][1] == CH and CH >= 2:
                # taper the final chunk so the post-DMA tail is short
                t_last, _ = chunks[-1]
                chunks[-1] = (t_last, CH - 1)
                chunks.append((t_last + CH - 1, 1))
            for ci, (t0, w) in enumerate(chunks):
                tb = bfp.tile([128, CH, H], BF16, tag="tokb")
                src = tok_d[t0 * 128 : (t0 + w) * 128, :].rearrange(
                    "(c p) h -> p c h", p=128
                )
                # f32 -> bf16 cast happens inside the DMA engine (SWDGE/Pool)
                nc.gpsimd.dma_start(out=tb[:, :w, : H // 2], in_=src[:, :, : H // 2])
                nc.gpsimd.dma_start(out=tb[:, :w, H // 2 :], in_=src[:, :, H // 2 :])
                rel = smallp.tile([128, CH], F32, tag="rel")
                nc.vector.tensor_tensor(
                    out=rel[:, :w],
                    in0=ids_f[:, t0 : t0 + w],
                    in1=base_s[:, g : g + 1].to_broadcast([128, w]),
                    op=mybir.AluOpType.subtract,
                )
                oh = ohp.tile([128, CH, 128], BF16, tag="oh")
                nc.vector.tensor_tensor(
                    out=oh[:, :w, :],
                    in0=rel[:, :w, None].to_broadcast([128, w, 128]),
                    in1=iota_s[:, None, :].to_broadcast([128, w, 128]),
                    op=mybir.AluOpType.is_equal,
                )
                for i in range(w):
                    first = ci == 0 and i == 0
                    last = ci == len(chunks) - 1 and i == w - 1
                    nc.tensor.matmul(
                        ps[:, :512], oh[:, i, :], tb[:, i, :512], start=first, stop=last
                    )
                    nc.tensor.matmul(
                        ps[:, 512:], oh[:, i, :], tb[:, i, 512:], start=first, stop=last
                    )
            # evacuate group (pipelined per 128-col block):
            # sums -> bf16 -> PE transpose -> FC matmul -> scale -> DMA out
            sums_bf = evacp.tile([128, H], BF16, tag="sums")
            pst = psT.tile([128, H], BF16, tag="psT", space="PSUM")
            sumsT = evacp.tile([128, H], BF16, tag="sumsT")
            psf = psF.tile([128, 16], F32, tag="psF", space="PSUM")
            for k in range(6):
                sl = slice(k * 128, (k + 1) * 128)
                nc.scalar.copy(sums_bf[:, sl], ps[:, sl])
                nc.tensor.transpose(pst[:, sl], sums_bf[:, sl], ident[:])
                nc.vector.tensor_copy(sumsT[:, sl], pst[:, sl])
                nc.tensor.matmul(
                    psf[:],
                    sumsT[:, sl],
                    fcw_s[:, k, :],
                    start=(k == 0),
                    stop=(k == 5),
                )
            lg = smallp.tile([128, 16], F32, tag="lg")
            nc.vector.tensor_scalar(
                out=lg[:],
                in0=psf[:],
                scalar1=invc[:, g : g + 1],
                scalar2=None,
                op0=mybir.AluOpType.mult,
            )
            nc.sync.dma_start(out=out_d[g], in_=lg[:])

    nc.compile()
    return nc


def _build_general():
    """Static full-sweep fallback: every tile vs all 13 clause windows.
    Correct for arbitrary seg_ids; slower (PE-bound). Collective-free:
    outputs all 13 window blocks, host adds the 8 per-core copies."""
    nc = bacc.Bacc(
        "TRN2",
        target_bir_lowering=False,
        debug=False,
        enable_asserts=False,
        num_devices=CORES,
    )
    NW = NCP // 128  # 13
    tok_d = nc.dram_tensor("tok", [TPC, H], F32, kind="ExternalInput")
    ids_d = nc.dram_tensor("ids", [128, NT], I32, kind="ExternalInput")
    cwg_d = nc.dram_tensor("cwg", [128, NW], F32, kind="ExternalInput")
    iota_d = nc.dram_tensor("iota", [128, 128], F32, kind="ExternalInput")
    fcw_d = nc.dram_tensor("fcw", [128, 6, 16], F32, kind="ExternalInput")
    out_d = nc.dram_tensor("out", [NW, 128, 16], F32, kind="ExternalOutput")

    from contextlib import ExitStack

    with tile.TileContext(nc) as tc, ExitStack() as ctx:
        cpool = ctx.enter_context(tc.tile_pool(name="const", bufs=1))
        iota_s = cpool.tile([128, 128], F32)
        nc.sync.dma_start(out=iota_s[:], in_=iota_d[:])
        ident = cpool.tile([128, 128], BF16)
        make_identity(nc, ident[:])
        fcw_f = cpool.tile([128, 6, 16], F32)
        nc.sync.dma_start(out=fcw_f[:], in_=fcw_d[:])
        fcw_s = cpool.tile([128, 6, 16], BF16)
        nc.vector.tensor_copy(fcw_s[:], fcw_f[:])
        ids_i = cpool.tile([128, NT], I32)
        nc.sync.dma_start(out=ids_i[:], in_=ids_d[:])
        ids_f = cpool.tile([128, NT], F32)
        nc.vector.tensor_copy(ids_f[:], ids_i[:])
        cwg_s = cpool.tile([128, NW], F32)
        nc.sync.dma_start(out=cwg_s[:], in_=cwg_d[:])
        invc = cpool.tile([128, NW], F32)
        nc.vector.reciprocal(invc[:], cwg_s[:])
        # all tokens resident in bf16: 6 MB of SBUF
        allb = cpool.tile([128, NT, H], BF16)

        tokp = ctx.enter_context(tc.tile_pool(name="tokf", bufs=6))
        ohp = ctx.enter_context(tc.tile_pool(name="oh", bufs=4))
        smallp = ctx.enter_context(tc.tile_pool(name="small", bufs=8))
        evacp = ctx.enter_context(tc.tile_pool(name="evac", bufs=2))
        psA = ctx.enter_context(tc.tile_pool(name="psA", bufs=2, space="PSUM"))
        psT = ctx.enter_context(tc.tile_pool(name="psT", bufs=2, space="PSUM"))
        psF = ctx.enter_context(tc.tile_pool(name="psF", bufs=2, space="PSUM"))

        for t in range(NT):
            tf = tokp.tile([128, H], F32, tag="tokf")
            nc.sync.dma_start(out=tf[:], in_=tok_d[t * 128 : (t + 1) * 128, :])
            nc.vector.tensor_copy(allb[:, t, :], tf[:])
        for w in range(NW):
            ps = psA.tile([128, H], F32, tag="psA", space="PSUM")
            for t in range(NT):
                rel = smallp.tile([128, 1], F32, tag="rel")
                nc.vector.tensor_scalar(
                    out=rel[:],
                    in0=ids_f[:, t : t + 1],
                    scalar1=float(w * 128),
                    scalar2=None,
                    op0=mybir.AluOpType.subtract,
                )
                oh = ohp.tile([128, 128], BF16, tag="oh")
                nc.vector.tensor_scalar(
                    out=oh[:],
                    in0=iota_s[:],
                    scalar1=rel[:, :1],
                    scalar2=None,
                    op0=mybir.AluOpType.is_equal,
                )
                nc.tensor.matmul(
                    ps[:, :512], oh[:], allb[:, t, :512],
                    start=(t == 0), stop=(t == NT - 1),
                )
                nc.tensor.matmul(
                    ps[:, 512:], oh[:], allb[:, t, 512:],
                    start=(t == 0), stop=(t == NT - 1),
                )
            sums_bf = evacp.tile([128, H], BF16, tag="sums")
            nc.scalar.copy(sums_bf[:], ps[:])
            pst = psT.tile([128, H], BF16, tag="psT", space="PSUM")
            for k in range(6):
                nc.tensor.transpose(
                    pst[:, k * 128 : (k + 1) * 128],
                    sums_bf[:, k * 128 : (k + 1) * 128],
                    ident[:],
                )
            sumsT = evacp.tile([128, H], BF16, tag="sumsT")
            nc.vector.tensor_copy(sumsT[:], pst[:])
            psf = psF.tile([128, 16], F32, tag="psF", space="PSUM")
            for k in range(6):
                nc.tensor.matmul(
                    psf[:],
                    sumsT[:, k * 128 : (k + 1) * 128],
                    fcw_s[:, k, :],
                    start=(k == 0),
                    stop=(k == 5),
                )
            lg = smallp.tile([128, 16], F32, tag="lg")
            nc.vector.tensor_scalar(
                out=lg[:],
                in0=psf[:],
                scalar1=invc[:, w : w + 1],
                scalar2=None,
                op0=mybir.AluOpType.mult,
            )
            nc.sync.dma_start(out=out_d[w], in_=lg[:])

    nc.compile()
    return nc


def kernel(
    sequence_output,
    fc_w,
    fc_b,
    clause_counts,
    seg_ids,
    n_clauses=NC,
    _ch=4,
):
    tok = np.ascontiguousarray(np.asarray(sequence_output, dtype=np.float32)).reshape(
        B * S, H
    )
    fc_w = np.asarray(fc_w, dtype=np.float32)
    fc_b = np.asarray(fc_b, dtype=np.float32)
    counts = np.asarray(clause_counts, dtype=np.float32)
    seg = np.asarray(seg_ids, dtype=np.int32).reshape(-1)

    counts_pad = np.ones(NCP, dtype=np.float32)
    counts_pad[:NC] = counts
    iota = np.broadcast_to(
        np.arange(128, dtype=np.float32)[None, :], (128, 128)
    ).copy()
    # fcw[p, k, j] = fc_w[j, k*128+p]
    fcw = np.ascontiguousarray(fc_w.reshape(16, 6, 128).transpose(2, 1, 0))

    R, bases = _choose_groups(seg)
    in_maps = []
    if R is not None:
        G = NT // R
        nc = _build_fast(R, CH=_ch)
        for c in range(CORES):
            seg_c = seg[c * TPC : (c + 1) * TPC]
            ids = np.ascontiguousarray(seg_c.reshape(NT, 128).T)  # [128, NT]
            base = np.broadcast_to(
                bases[c].astype(np.float32)[None, :], (128, G)
            ).copy()
            idx = (
                bases[c][None, :].astype(np.int64) + np.arange(128)[:, None]
            ).astype(np.int32)
            cw = counts_pad[idx]
            in_maps.append(
                {
                    "tok": tok[c * TPC : (c + 1) * TPC],
                    "ids": ids,
                    "base": base,
                    "cw": np.ascontiguousarray(cw),
                    "iota": iota,
                    "fcw": fcw,
                }
            )
    else:
        nc = _build_general()
        NW = NCP // 128
        cwg = np.ascontiguousarray(counts_pad.reshape(NW, 128).T)  # [128, NW]
        for c in range(CORES):
            seg_c = seg[c * TPC : (c + 1) * TPC]
            ids = np.ascontiguousarray(seg_c.reshape(NT, 128).T)
            in_maps.append(
                {
                    "tok": tok[c * TPC : (c + 1) * TPC],
                    "ids": ids,
                    "cwg": cwg,
                    "iota": iota,
                    "fcw": fcw,
                }
            )

    import time

    t0 = time.perf_counter()
    res = bass_utils.run_bass_kernel_spmd(
        nc, in_maps, core_ids=list(range(CORES)), trace=False
    )
    t1 = time.perf_counter()
    LAST_EXEC_INFO.clear()
    LAST_EXEC_INFO.update(
        {"wall_s": t1 - t0, "R": R, "nc": nc, "in_maps": in_maps, "res": res}
    )

    # host-side unshard: overlap-add the per-core window blocks, bias once
    acc = np.zeros((NCP, 16), dtype=np.float64)
    if R is not None:
        for c in range(CORES):
            blocks = res.results[c]["out"]  # [G, 128, 16]
            for g in range(G):
                b = int(bases[c][g])
                acc[b : b + 128] += blocks[g]
    else:
        NW = NCP // 128
        for c in range(CORES):
            blocks = res.results[c]["out"]  # [NW, 128, 16]
            acc += blocks.reshape(NCP, 16)
    out = acc[:NC].astype(np.float32) + fc_b[None, :]
    return out


# revision 38
# speedup vs baseline: 1.0086x; 1.0086x over previous
"""Trainium2 Bass kernel for nn_Bert_AvgPooling (segment_reduce + mean + FC).

reference semantics:
    tokens = sequence_output.reshape(B*S, H)              # [32768, 768]
    sums   = segment_sum(tokens, seg_ids, 1537)           # sentinel id 1536
    mean   = sums[:1536] / clause_counts[:, None]
    logits = mean @ fc_w.T + fc_b                         # [1536, 16]

Strategy (8 cores, data parallel over tokens, NO collective):
  - Each core streams its 4096 tokens (12 MB f32) from HBM; the f32->bf16
    cast happens inside the SWDGE DMA engine.
  - Per 128-token tile a one-hot matrix (token x clause-window) is built on
    DVE with is_equal against an iota row, then PE matmul accumulates
    window sums in PSUM:  psum[c, h] += onehot.T @ tokens.
  - Clause ids are monotone over the token stream, so each group of R=16
    tiles shares one 128-wide clause window (host-verified; static 12-window
    full-sweep fallback for unsorted ids).
  - Per group: PSUM sums -> bf16 -> PE transpose -> FC matmul (768->16)
    -> scale by 1/count -> DMA the [128,16] window block to DRAM.
  - The host overlap-adds the 16 window blocks (windows of adjacent
    cores/groups share at most one boundary clause) and adds the bias once.
    This removes the ReduceScatter + arrival barrier (~65us) entirely.
"""

import sys

for _p in ("/opt/trn_rl_repo", "/opt/trn_rl_repo/concourse"):
    if _p not in sys.path:
        sys.path.insert(0, _p)

import numpy as np

import concourse.bacc as bacc
import concourse.bass as bass
import concourse.mybir as mybir
import concourse.tile as tile
from concourse import bass_utils
from concourse.masks import make_identity

F32 = mybir.dt.float32
F32R = mybir.dt.float32r
BF16 = mybir.dt.bfloat16
I32 = mybir.dt.int32

B, S, H, NC = 64, 512, 768, 1536
CORES = 8
TPC = B * S // CORES  # tokens per core = 4096
NT = TPC // 128  # token tiles per core = 32
NCP = 1664  # padded clause rows (13 * 128)

# module-level stash for benchmarking (test.py reads these)
LAST_EXEC_INFO = {}


def _choose_groups(seg_flat):
    """Pick largest R in {32,16,8,4,2,1} s.t. every (core, group-of-R-tiles)
    clause-id span fits in a 128-wide window. Returns (R, bases[CORES][G])
    or (None, None) if even R=1 fails (=> general fallback path)."""
    ids = seg_flat.reshape(CORES, NT, 128)
    for R in (32, 16, 8, 4, 2, 1):
        G = NT // R
        bases = np.zeros((CORES, G), dtype=np.int64)
        ok = True
        for c in range(CORES):
            for g in range(G):
                grp = ids[c, g * R : (g + 1) * R].reshape(-1)
                real = grp[grp < NC]
                if real.size == 0:
                    bases[c, g] = 0
                    continue
                lo, hi = int(real.min()), int(real.max())
                if hi - lo > 127:
                    ok = False
                    break
                # clamp so the 128-row window stays inside the padded range
                bases[c, g] = min(lo, NCP - 128)
            if not ok:
                break
        if ok:
            return R, bases
    return None, None


def _choose_groups_gathered(gids, NTg):
    """Uniform R (tiles per group) for gathered ids [CORES][128, NTg] with pad
    id >= NC. Largest R whose per-(core, group) span fits a 128 window."""
    for R in range(NTg, 0, -1):
        G = -(-NTg // R)
        bases = np.zeros((CORES, G), dtype=np.int64)
        ok = True
        for c in range(CORES):
            for g in range(G):
                grp = gids[c][:, g * R : min((g + 1) * R, NTg)].reshape(-1)
                real = grp[grp < NC]
                if real.size == 0:
                    bases[c, g] = 0
                    continue
                lo, hi = int(real.min()), int(real.max())
                if hi - lo > 127:
                    ok = False
                    break
                bases[c, g] = min(lo, NCP - 128)
            if not ok:
                break
        if ok:
            return R, bases
    return None, None


def _build_gather(NTg, R, CH=4, evacmix=False, warm=40, pools=None):
    """Gather-mode fast path: indirect DMA pulls ONLY the masked token rows
    (~75% of the stream), casting f32->bf16 in the DMA engine. One indirect
    DMA per 128-row tile with a [128,1] row-index column."""
    G = -(-NTg // R)
    nc = bacc.Bacc(
        "TRN2",
        target_bir_lowering=False,
        debug=False,
        enable_asserts=False,
        num_devices=CORES,
    )
    OFF_BASE = NTg
    OFF_CW = OFF_BASE + G
    CF_COLS = OFF_CW + G
    OFF_IDENT = 128
    OFF_FCW = OFF_IDENT + 128
    CB_COLS = OFF_FCW + 96

    tok_d = nc.dram_tensor("tok", [TPC, H], F32, kind="ExternalInput")
    idx_d = nc.dram_tensor("idx", [128, NTg], I32, kind="ExternalInput")
    cstf_d = nc.dram_tensor("cstf", [128, CF_COLS], F32, kind="ExternalInput")
    cstb_d = nc.dram_tensor("cstb", [128, CB_COLS], BF16, kind="ExternalInput")
    out_d = nc.dram_tensor("out", [G, 128, 16], F32, kind="ExternalOutput")

    from contextlib import ExitStack

    with tile.TileContext(nc) as tc, ExitStack() as ctx:
        cpool = ctx.enter_context(tc.tile_pool(name="const", bufs=1))
        with tc.high_priority():
            cstf_s = cpool.tile([128, CF_COLS], F32)
            nc.sync.dma_start(out=cstf_s[:], in_=cstf_d[:])
            cstb_s = cpool.tile([128, CB_COLS], BF16)
            nc.sync.dma_start(out=cstb_s[:], in_=cstb_d[:])
            idx_s = cpool.tile([128, NTg], I32)
            nc.sync.dma_start(out=idx_s[:], in_=idx_d[:])
            iota_s = cpool.tile([128, 128], F32)
            nc.vector.tensor_copy(iota_s[:], cstb_s[:, 0:128])
            ids_f = cpool.tile([128, NTg], F32)
            nc.vector.tensor_copy(ids_f[:], cstf_s[:, 0:NTg])
            base_s = cpool.tile([128, G], F32)
            nc.vector.tensor_copy(base_s[:], cstf_s[:, OFF_BASE : OFF_BASE + G])
            ident = cpool.tile([128, 128], BF16)
            nc.vector.tensor_copy(ident[:], cstb_s[:, OFF_IDENT : OFF_IDENT + 128])
            fcw_s = cpool.tile([128, 96], BF16)
            nc.vector.tensor_copy(fcw_s[:], cstb_s[:, OFF_FCW : OFF_FCW + 96])
            invc = cpool.tile([128, G], F32)
            nc.vector.reciprocal(invc[:], cstf_s[:, OFF_CW : OFF_CW + G])

        P = pools or {}
        bfp = ctx.enter_context(
            tc.tile_pool(name="tokb", bufs=P.get("bfp", max(4, 48 // CH)))
        )
        ohp = ctx.enter_context(tc.tile_pool(name="oh", bufs=P.get("ohp", 8)))
        smallp = ctx.enter_context(tc.tile_pool(name="small", bufs=P.get("smallp", 8)))
        evacp = ctx.enter_context(tc.tile_pool(name="evac", bufs=P.get("evacp", 2)))
        psA = ctx.enter_context(tc.tile_pool(name="psA", bufs=2, space="PSUM"))
        psT = ctx.enter_context(tc.tile_pool(name="psT", bufs=2, space="PSUM"))
        psF = ctx.enter_context(tc.tile_pool(name="psF", bufs=2, space="PSUM"))

        ps_tiles = [
            psA.tile([128, H], F32, tag="psA", space="PSUM", name=f"psa{g}")
            for g in range(G)
        ]
        if warm:
            with tc.high_priority():
                for i in range(warm):
                    nc.tensor.matmul(
                        ps_tiles[0][:, :128], ident[:], ident[:],
                        start=(i == 0), stop=(i == warm - 1),
                    )

        for g in range(G):
            ps = ps_tiles[g]
            tlist = list(range(g * R, min((g + 1) * R, NTg)))
            chunks = [tlist[c0 : c0 + CH] for c0 in range(0, len(tlist), CH)]
            if g == G - 1 and len(chunks[-1]) == CH and CH >= 2:
                last = chunks[-1]
                chunks[-1] = last[:-1]
                chunks.append(last[-1:])
            for ci, tchunk in enumerate(chunks):
                w = len(tchunk)
                t0 = tchunk[0]
                tb = bfp.tile([128, CH, H], BF16, tag="tokb")
                for i, t in enumerate(tchunk):
                    # bf16 cast happens inside the SWDGE engine during gather
                    nc.gpsimd.indirect_dma_start(
                        out=tb[:, i, :],
                        out_offset=None,
                        in_=tok_d[:],
                        in_offset=bass.IndirectOffsetOnAxis(
                            ap=idx_s[:, t : t + 1], axis=0
                        ),
                    )
                rel = smallp.tile([128, CH], F32, tag="rel")
                nc.vector.tensor_tensor(
                    out=rel[:, :w],
                    in0=ids_f[:, t0 : t0 + w],
                    in1=base_s[:, g : g + 1].to_broadcast([128, w]),
                    op=mybir.AluOpType.subtract,
                )
                oh = ohp.tile([128, CH, 128], BF16, tag="oh")
                nc.vector.tensor_tensor(
                    out=oh[:, :w, :],
                    in0=rel[:, :w, None].to_broadcast([128, w, 128]),
                    in1=iota_s[:, None, :].to_broadcast([128, w, 128]),
                    op=mybir.AluOpType.is_equal,
                )
                for i in range(w):
                    first = ci == 0 and i == 0
                    last = ci == len(chunks) - 1 and i == w - 1
                    nc.tensor.matmul(
                        ps[:, :512], oh[:, i, :], tb[:, i, :512],
                        start=first, stop=last,
                    )
                    nc.tensor.matmul(
                        ps[:, 512:], oh[:, i, :], tb[:, i, 512:],
                        start=first, stop=last,
                    )
            sums_bf = evacp.tile([128, H], BF16, tag="sums")
            pst = psT.tile([128, H], BF16, tag="psT", space="PSUM")
            sumsT = evacp.tile([128, H], BF16, tag="sumsT")
            psf = psF.tile([128, 16], F32, tag="psF", space="PSUM")
            for k in range(6):
                sl = slice(k * 128, (k + 1) * 128)
                if evacmix and g == G - 1 and k % 2 == 1:
                    nc.vector.tensor_copy(sums_bf[:, sl], ps[:, sl])
                else:
                    nc.scalar.copy(sums_bf[:, sl], ps[:, sl])
                nc.tensor.transpose(pst[:, sl], sums_bf[:, sl], ident[:])
                nc.vector.tensor_copy(sumsT[:, sl], pst[:, sl])
                nc.tensor.matmul(
                    psf[:],
                    sumsT[:, sl],
                    fcw_s[:, k * 16 : (k + 1) * 16],
                    start=(k == 0),
                    stop=(k == 5),
                )
            lg = smallp.tile([128, 16], F32, tag="lg")
            nc.vector.tensor_scalar(
                out=lg[:],
                in0=psf[:],
                scalar1=invc[:, g : g + 1],
                scalar2=None,
                op0=mybir.AluOpType.mult,
            )
            nc.sync.dma_start(out=out_d[g], in_=lg[:])

    nc.compile()
    return nc


def _build_fast(R, CH=4, split=True, early=True, evacmix=True, warm=0, pools=None,
                f32=False, dmat=True):
    """Collective-free grouped-window path. Same program for all cores;
    per-core data differences come in through input tensors."""
    G = NT // R
    nc = bacc.Bacc(
        "TRN2",
        target_bir_lowering=False,
        debug=False,
        enable_asserts=False,
        num_devices=CORES,
    )
    # packed consts, two parallel DMAs:
    #   f32 block:  ids | base | cw        (values that need full precision)
    #   bf16 block: iota | ident | fcw     (small-int / already-bf16 payloads)
    OFF_BASE = NT
    OFF_CW = OFF_BASE + G
    CF_COLS = OFF_CW + G
    OFF_IDENT = 128
    OFF_FCW = OFF_IDENT + 128
    CB_COLS = OFF_FCW + 96

    tok_d = nc.dram_tensor("tok", [TPC, H], F32, kind="ExternalInput")
    cstf_d = nc.dram_tensor("cstf", [128, CF_COLS], F32, kind="ExternalInput")
    cstb_d = nc.dram_tensor("cstb", [128, CB_COLS], BF16, kind="ExternalInput")
    out_d = nc.dram_tensor("out", [G, 128, 16], F32, kind="ExternalOutput")

    from contextlib import ExitStack

    with tile.TileContext(nc) as tc, ExitStack() as ctx:
        cpool = ctx.enter_context(tc.tile_pool(name="const", bufs=1))
        with tc.high_priority():
            cstf_s = cpool.tile([128, CF_COLS], F32)
            nc.sync.dma_start(out=cstf_s[:], in_=cstf_d[:])
            cstb_s = cpool.tile([128, CB_COLS], BF16)
            nc.sync.dma_start(out=cstb_s[:], in_=cstb_d[:])
            iota_s = cpool.tile([128, 128], F32)
            nc.vector.tensor_copy(iota_s[:], cstb_s[:, 0:128])
            ids_f = cpool.tile([128, NT], F32)
            nc.vector.tensor_copy(ids_f[:], cstf_s[:, 0:NT])
            base_s = cpool.tile([128, G], F32)
            nc.vector.tensor_copy(base_s[:], cstf_s[:, OFF_BASE : OFF_BASE + G])
            ident = cpool.tile([128, 128], BF16)
            nc.vector.tensor_copy(ident[:], cstb_s[:, OFF_IDENT : OFF_IDENT + 128])
            fcw_s = cpool.tile([128, 96], BF16)
            nc.vector.tensor_copy(fcw_s[:], cstb_s[:, OFF_FCW : OFF_FCW + 96])
            invc = cpool.tile([128, G], F32)
            nc.vector.reciprocal(invc[:], cstf_s[:, OFF_CW : OFF_CW + G])

        P = pools or {}
        bfp = ctx.enter_context(
            tc.tile_pool(name="tokb", bufs=P.get("bfp", max(4, 48 // CH)))
        )
        ohp = ctx.enter_context(tc.tile_pool(name="oh", bufs=P.get("ohp", 8)))
        smallp = ctx.enter_context(tc.tile_pool(name="small", bufs=P.get("smallp", 8)))
        evacp = ctx.enter_context(tc.tile_pool(name="evac", bufs=P.get("evacp", 2)))
        psA = ctx.enter_context(tc.tile_pool(name="psA", bufs=2, space="PSUM"))
        psT = ctx.enter_context(tc.tile_pool(name="psT", bufs=2, space="PSUM"))
        psF = ctx.enter_context(tc.tile_pool(name="psF", bufs=2, space="PSUM"))

        ps_tiles = [
            psA.tile([128, H], F32, tag="psA", space="PSUM", name=f"psa{g}")
            for g in range(G)
        ]
        if warm:
            # PE HAM clock-gate warmup: the PE boots throttled to half rate and
            # needs ~4us of sustained matmul activity to reach full rate.  The
            # first ~9us of the program are DMA/const setup with an idle PE, so
            # spin it on dummy matmuls.  Target: group 0's PSUM tile — the real
            # accumulation begins with start=True, which clears it.
            with tc.high_priority():
                for i in range(warm):
                    nc.tensor.matmul(
                        ps_tiles[0][:, :128], ident[:], ident[:],
                        start=(i == 0), stop=(i == warm - 1),
                    )

        for g in range(G):
            ps = ps_tiles[g]
            chunks = [(g * R + c0, min(CH, R - c0)) for c0 in range(0, R, CH)]
            if g == G - 1 and chunks[-1][1] == CH and CH >= 2:
                # taper the final chunk so the post-DMA tail is short
                t_last, _ = chunks[-1]
                chunks[-1] = (t_last, CH - 1)
                chunks.append((t_last + CH - 1, 1))
            for ci, (t0, w) in enumerate(chunks):
                tb = bfp.tile([128, CH, H], F32 if f32 else BF16, tag="tokb")
                src = tok_d[t0 * 128 : (t0 + w) * 128, :].rearrange(
                    "(c p) h -> p c h", p=128
                )
                import contextlib

                prio = tc.high_priority() if early else contextlib.nullcontext()
                with prio:
                    if f32:
                        # raw f32 via HWDGE; PE streams it as float32r at full
                        # rate (1 cycle/row for moving N >= 256)
                        nc.sync.dma_start(out=tb[:, :w, :], in_=src)
                    elif split:
                        # f32 -> bf16 cast happens inside the DMA engine (SWDGE)
                        nc.gpsimd.dma_start(
                            out=tb[:, :w, : H // 2], in_=src[:, :, : H // 2]
                        )
                        nc.gpsimd.dma_start(
                            out=tb[:, :w, H // 2 :], in_=src[:, :, H // 2 :]
                        )
                    else:
                        nc.gpsimd.dma_start(out=tb[:, :w, :], in_=src)
                rel = smallp.tile([128, CH], F32, tag="rel")
                nc.vector.tensor_tensor(
                    out=rel[:, :w],
                    in0=ids_f[:, t0 : t0 + w],
                    in1=base_s[:, g : g + 1].to_broadcast([128, w]),
                    op=mybir.AluOpType.subtract,
                )
                oh = ohp.tile([128, CH, 128], F32 if f32 else BF16, tag="oh")
                nc.vector.tensor_tensor(
                    out=oh[:, :w, :],
                    in0=rel[:, :w, None].to_broadcast([128, w, 128]),
                    in1=iota_s[:, None, :].to_broadcast([128, w, 128]),
                    op=mybir.AluOpType.is_equal,
                )
                for i in range(w):
                    first = ci == 0 and i == 0
                    last = ci == len(chunks) - 1 and i == w - 1
                    mv0 = tb[:, i, :512]
                    mv1 = tb[:, i, 512:]
                    st = oh[:, i, :]
                    if f32:
                        mv0 = mv0.bitcast(F32R)
                        mv1 = mv1.bitcast(F32R)
                        st = st.bitcast(F32R)
                    nc.tensor.matmul(
                        ps[:, :512], st, mv0, start=first, stop=last
                    )
                    nc.tensor.matmul(
                        ps[:, 512:], st, mv1, start=first, stop=last
                    )
            # evacuate group (pipelined per 128-col block):
            # sums -> bf16 -> transpose -> FC matmul -> scale -> DMA out
            sums_bf = evacp.tile([128, H], BF16, tag="sums")
            sumsT = evacp.tile([128, H], BF16, tag="sumsT")
            psf = psF.tile([128, 16], F32, tag="psF", space="PSUM")
            if dmat:
                pst = None
            else:
                pst = psT.tile([128, H], BF16, tag="psT", space="PSUM")
            for k in range(6):
                sl = slice(k * 128, (k + 1) * 128)
                # mid-stream groups: keep DVE free for one-hot building; final
                # group: DVE is idle, split copies across ACT+DVE
                if evacmix and g == G - 1 and k % 2 == 1:
                    nc.vector.tensor_copy(sums_bf[:, sl], ps[:, sl])
                else:
                    nc.scalar.copy(sums_bf[:, sl], ps[:, sl])
                if dmat:
                    # transpose on the DMA XBAR (idle at the tail) instead of
                    # the PE, freeing ~4 tensor-engine ops per block
                    nc.sync.dma_start(
                        out=sumsT[:, sl], in_=sums_bf[:, sl], transpose=True
                    )
                else:
                    nc.tensor.transpose(pst[:, sl], sums_bf[:, sl], ident[:])
                    nc.vector.tensor_copy(sumsT[:, sl], pst[:, sl])
                nc.tensor.matmul(
                    psf[:],
                    sumsT[:, sl],
                    fcw_s[:, k * 16 : (k + 1) * 16],
                    start=(k == 0),
                    stop=(k == 5),
                )
            lg = smallp.tile([128, 16], F32, tag="lg")
            nc.vector.tensor_scalar(
                out=lg[:],
                in0=psf[:],
                scalar1=invc[:, g : g + 1],
                scalar2=None,
                op0=mybir.AluOpType.mult,
            )
            nc.sync.dma_start(out=out_d[g], in_=lg[:])

    nc.compile()
    return nc


def _build_general():
    """Static full-sweep fallback: every tile vs all 13 clause windows.
    Correct for arbitrary seg_ids; slower (PE-bound). Collective-free:
    outputs all 13 window blocks, host adds the 8 per-core copies."""
    nc = bacc.Bacc(
        "TRN2",
        target_bir_lowering=False,
        debug=False,
        enable_asserts=False,
        num_devices=CORES,
    )
    NW = NCP // 128  # 13
    tok_d = nc.dram_tensor("tok", [TPC, H], F32, kind="ExternalInput")
    ids_d = nc.dram_tensor("ids", [128, NT], I32, kind="ExternalInput")
    cwg_d = nc.dram_tensor("cwg", [128, NW], F32, kind="ExternalInput")
    iota_d = nc.dram_tensor("iota", [128, 128], F32, kind="ExternalInput")
    fcw_d = nc.dram_tensor("fcw", [128, 6, 16], F32, kind="ExternalInput")
    out_d = nc.dram_tensor("out", [NW, 128, 16], F32, kind="ExternalOutput")

    from contextlib import ExitStack

    with tile.TileContext(nc) as tc, ExitStack() as ctx:
        cpool = ctx.enter_context(tc.tile_pool(name="const", bufs=1))
        iota_s = cpool.tile([128, 128], F32)
        nc.sync.dma_start(out=iota_s[:], in_=iota_d[:])
        ident = cpool.tile([128, 128], BF16)
        make_identity(nc, ident[:])
        fcw_f = cpool.tile([128, 6, 16], F32)
        nc.sync.dma_start(out=fcw_f[:], in_=fcw_d[:])
        fcw_s = cpool.tile([128, 6, 16], BF16)
        nc.vector.tensor_copy(fcw_s[:], fcw_f[:])
        ids_i = cpool.tile([128, NT], I32)
        nc.sync.dma_start(out=ids_i[:], in_=ids_d[:])
        ids_f = cpool.tile([128, NT], F32)
        nc.vector.tensor_copy(ids_f[:], ids_i[:])
        cwg_s = cpool.tile([128, NW], F32)
        nc.sync.dma_start(out=cwg_s[:], in_=cwg_d[:])
        invc = cpool.tile([128, NW], F32)
        nc.vector.reciprocal(invc[:], cwg_s[:])
        # all tokens resident in bf16: 6 MB of SBUF
        allb = cpool.tile([128, NT, H], BF16)

        tokp = ctx.enter_context(tc.tile_pool(name="tokf", bufs=6))
        ohp = ctx.enter_context(tc.tile_pool(name="oh", bufs=4))
        smallp = ctx.enter_context(tc.tile_pool(name="small", bufs=8))
        evacp = ctx.enter_context(tc.tile_pool(name="evac", bufs=2))
        psA = ctx.enter_context(tc.tile_pool(name="psA", bufs=2, space="PSUM"))
        psT = ctx.enter_context(tc.tile_pool(name="psT", bufs=2, space="PSUM"))
        psF = ctx.enter_context(tc.tile_pool(name="psF", bufs=2, space="PSUM"))

        for t in range(NT):
            tf = tokp.tile([128, H], F32, tag="tokf")
            nc.sync.dma_start(out=tf[:], in_=tok_d[t * 128 : (t + 1) * 128, :])
            nc.vector.tensor_copy(allb[:, t, :], tf[:])
        for w in range(NW):
            ps = psA.tile([128, H], F32, tag="psA", space="PSUM")
            for t in range(NT):
                rel = smallp.tile([128, 1], F32, tag="rel")
                nc.vector.tensor_scalar(
                    out=rel[:],
                    in0=ids_f[:, t : t + 1],
                    scalar1=float(w * 128),
                    scalar2=None,
                    op0=mybir.AluOpType.subtract,
                )
                oh = ohp.tile([128, 128], BF16, tag="oh")
                nc.vector.tensor_scalar(
                    out=oh[:],
                    in0=iota_s[:],
                    scalar1=rel[:, :1],
                    scalar2=None,
                    op0=mybir.AluOpType.is_equal,
                )
                nc.tensor.matmul(
                    ps[:, :512], oh[:], allb[:, t, :512],
                    start=(t == 0), stop=(t == NT - 1),
                )
                nc.tensor.matmul(
                    ps[:, 512:], oh[:], allb[:, t, 512:],
                    start=(t == 0), stop=(t == NT - 1),
                )
            sums_bf = evacp.tile([128, H], BF16, tag="sums")
            nc.scalar.copy(sums_bf[:], ps[:])
            pst = psT.tile([128, H], BF16, tag="psT", space="PSUM")
            for k in range(6):
                nc.tensor.transpose(
                    pst[:, k * 128 : (k + 1) * 128],
                    sums_bf[:, k * 128 : (k + 1) * 128],
                    ident[:],
                )
            sumsT = evacp.tile([128, H], BF16, tag="sumsT")
            nc.vector.tensor_copy(sumsT[:], pst[:])
            psf = psF.tile([128, 16], F32, tag="psF", space="PSUM")
            for k in range(6):
                nc.tensor.matmul(
                    psf[:],
                    sumsT[:, k * 128 : (k + 1) * 128],
                    fcw_s[:, k, :],
                    start=(k == 0),
                    stop=(k == 5),
                )
            lg = smallp.tile([128, 16], F32, tag="lg")
            nc.vector.tensor_scalar(
                out=lg[:],
                in0=psf[:],
                scalar1=invc[:, w : w + 1],
                scalar2=None,
                op0=mybir.AluOpType.mult,
            )
            nc.sync.dma_start(out=out_d[w], in_=lg[:])

    nc.compile()
    return nc


def kernel(
    sequence_output,
    fc_w,
    fc_b,
    clause_counts,
    seg_ids,
    n_clauses=NC,
    _ch=4,
    _split=False,
    _early=True,
    _evacmix=False,
    _warm=40,
    _pools=None,
    _f32=False,
    _dmat=False,
    _gather=False,
):
    tok = np.ascontiguousarray(np.asarray(sequence_output, dtype=np.float32)).reshape(
        B * S, H
    )
    fc_w = np.asarray(fc_w, dtype=np.float32)
    fc_b = np.asarray(fc_b, dtype=np.float32)
    counts = np.asarray(clause_counts, dtype=np.float32)
    seg = np.asarray(seg_ids, dtype=np.int32).reshape(-1)

    counts_pad = np.ones(NCP, dtype=np.float32)
    counts_pad[:NC] = counts
    iota = np.broadcast_to(
        np.arange(128, dtype=np.float32)[None, :], (128, 128)
    ).copy()
    # fcw[p, k, j] = fc_w[j, k*128+p]
    fcw = np.ascontiguousarray(fc_w.reshape(16, 6, 128).transpose(2, 1, 0))

    in_maps = []
    mode = None
    if _gather:
        # gather mode: pull only the masked rows (cuts HBM reads ~25%)
        PAD_ID = 100000.0
        per_core = []
        for c in range(CORES):
            seg_c = seg[c * TPC : (c + 1) * TPC]
            pos = np.nonzero(seg_c < NC)[0].astype(np.int32)
            per_core.append((pos, seg_c[pos].astype(np.int64)))
        NTg = max(-(-len(p) // 128) for p, _ in per_core)
        idxs, gids = [], []
        for pos, pid in per_core:
            n = NTg * 128
            idx_pad = np.zeros(n, dtype=np.int32)
            idx_pad[: len(pos)] = pos
            gid_pad = np.full(n, PAD_ID, dtype=np.float64)
            gid_pad[: len(pid)] = pid
            idxs.append(np.ascontiguousarray(idx_pad.reshape(NTg, 128).T))
            gids.append(np.ascontiguousarray(gid_pad.reshape(NTg, 128).T))
        Rg, bases_g = _choose_groups_gathered(gids, NTg)
        if Rg is not None:
            mode = "gather"
            R, bases = Rg, bases_g
            G = -(-NTg // Rg)
    if mode is None:
        R, bases = _choose_groups(seg)
        if R is not None:
            mode = "fast"
            G = NT // R
        else:
            mode = "general"

    if mode == "gather":
        nc = _build_gather(
            NTg, R, CH=_ch, evacmix=_evacmix, warm=_warm, pools=_pools
        )
        import ml_dtypes

        ident128 = np.eye(128, dtype=np.float32)
        cstb = np.ascontiguousarray(
            np.concatenate([iota, ident128, fcw.reshape(128, 96)], axis=1).astype(
                ml_dtypes.bfloat16
            )
        )
        for c in range(CORES):
            base = np.broadcast_to(bases[c].astype(np.float32)[None, :], (128, G))
            idx2 = (
                bases[c][None, :].astype(np.int64) + np.arange(128)[:, None]
            ).astype(np.int32)
            cw = counts_pad[idx2]
            cstf = np.ascontiguousarray(
                np.concatenate(
                    [gids[c].astype(np.float32), base, cw], axis=1
                ).astype(np.float32)
            )
            in_maps.append(
                {
                    "tok": tok[c * TPC : (c + 1) * TPC],
                    "idx": idxs[c],
                    "cstf": cstf,
                    "cstb": cstb,
                }
            )
    elif mode == "fast":
        G = NT // R
        nc = _build_fast(
            R, CH=_ch, split=_split, early=_early, evacmix=_evacmix,
            warm=_warm, pools=_pools, f32=_f32, dmat=_dmat,
        )
        import ml_dtypes

        ident128 = np.eye(128, dtype=np.float32)
        # bf16 const block is identical on every core
        cstb = np.ascontiguousarray(
            np.concatenate([iota, ident128, fcw.reshape(128, 96)], axis=1).astype(
                ml_dtypes.bfloat16
            )
        )
        for c in range(CORES):
            seg_c = seg[c * TPC : (c + 1) * TPC]
            ids = seg_c.reshape(NT, 128).T.astype(np.float32)  # [128, NT]
            base = np.broadcast_to(
                bases[c].astype(np.float32)[None, :], (128, G)
            )
            idx = (
                bases[c][None, :].astype(np.int64) + np.arange(128)[:, None]
            ).astype(np.int32)
            cw = counts_pad[idx]
            # f32 const block: ids | base | cw
            cstf = np.ascontiguousarray(
                np.concatenate([ids, base, cw], axis=1).astype(np.float32)
            )
            in_maps.append(
                {
                    "tok": tok[c * TPC : (c + 1) * TPC],
                    "cstf": cstf,
                    "cstb": cstb,
                }
            )
    else:
        nc = _build_general()
        NW = NCP // 128
        cwg = np.ascontiguousarray(counts_pad.reshape(NW, 128).T)  # [128, NW]
        for c in range(CORES):
            seg_c = seg[c * TPC : (c + 1) * TPC]
            ids = np.ascontiguousarray(seg_c.reshape(NT, 128).T)
            in_maps.append(
                {
                    "tok": tok[c * TPC : (c + 1) * TPC],
                    "ids": ids,
                    "cwg": cwg,
                    "iota": iota,
                    "fcw": fcw,
                }
            )

    import time

    t0 = time.perf_counter()
    res = bass_utils.run_bass_kernel_spmd(
        nc, in_maps, core_ids=list(range(CORES)), trace=False
    )
    t1 = time.perf_counter()
    LAST_EXEC_INFO.clear()
    LAST_EXEC_INFO.update(
        {"wall_s": t1 - t0, "R": R, "mode": mode, "nc": nc, "in_maps": in_maps, "res": res}
    )

    # host-side unshard: overlap-add the per-core window blocks, bias once
    acc = np.zeros((NCP, 16), dtype=np.float64)
    if R is not None:
        for c in range(CORES):
            blocks = res.results[c]["out"]  # [G, 128, 16]
            for g in range(G):
                b = int(bases[c][g])
                acc[b : b + 128] += blocks[g]
    else:
        NW = NCP // 128
        for c in range(CORES):
            blocks = res.results[c]["out"]  # [NW, 128, 16]
            acc += blocks.reshape(NCP, 16)
    out = acc[:NC].astype(np.float32) + fc_b[None, :]
    return out
